# revision 13
# baseline (speedup 1.0000x reference)
"""Bass/Trainium2 kernel for nn_Attn_22814866276758.

Computation (reference):
    h = hidden[-1, 0]                            # [H]
    proj = enc @ W.T + b                         # [S, H]
    energies = proj @ h                          # [S]
    attn = softmax(energies)                     # [1, 1, S]

Algebraic collapse: energies = enc @ (W.T @ h) + (b @ h).  The constant
b @ h is uniform over S, so it cancels inside softmax.  The kernel is
therefore a memory-bound matvec over the 128 MB encoder_outputs plus a
global softmax.

Distribution (8 cores):
  - enc sharded over seq: each core owns [4096, 1024] (16 MB).
  - v = W.T @ h  (tiny) precomputed on host, replicated to all cores.
  - Each core: e[p, t] = dot(enc_row, v); the elementwise multiply runs
    on the DVE and the row reduction on the ACT engine (activation-Copy
    with accumulate), so the two passes overlap.  Local row index
    s = p*32 + t (p = SBUF partition).
  - Each core then computes per-partition online-softmax stats
    (m = row max, s = sum of exp(e-m)) and writes exp(e-m) plus the
    [128, 2] stats.  The global combine — max/sum over the 8*128 stats
    pairs and one scale per element — happens on the host during the
    unshard (an on-device all-gather of the same stats measured 23 us
    of RDH transfer + ~15 us of trigger latency for 8 KB, dwarfing the
    math it feeds).

Toolchain workarounds (this container's walrus build):
  - EVENT_SEMAPHORE_RANGE_CLEAR / DMA_QUEUE_RESET at Tile exit are
    rejected ("ISA wrong length") -> skipped (PatchedBass).
  - Sync waits on the terminal Drain are rejected ("Too many sync wait
    commands") -> moved onto EVSEM no-ops (PatchedTC).
  - Any instruction with >=2 sync waits is rejected -> waits hoisted
    onto EVSEM no-ops at BIR-JSON level (PatchedBass.to_json_bytes).
  - TensorTensorReduce opcode is unknown -> use mul + reduce instead.
"""

import json
from contextlib import ExitStack

import numpy as np

import concourse.bass as bass
import concourse.mybir as mybir
import concourse.tile as tile
from concourse.bass import SemaphoreHandle
from concourse.bass_utils import run_bass_kernel_spmd
from concourse.tile_sem_assignment import N_PROCS
from concourse.vector_clock import ScopedClock, VectorClock

SEQ = 32768
HID = 1024
NCORES = 8
SHARD = SEQ // NCORES  # 4096
P = 128  # SBUF partitions
TCOLS = SHARD // P  # 32 energy columns per core; s_local = p*TCOLS + t
TPD = 4  # seq-columns per DMA: tile = [128, TPD, 1024] = 2 MB
NDMA = TCOLS // TPD
F32 = mybir.dt.float32

# test.py pokes these to get a profiled run; harness path keeps defaults.
TRACE = {"on": False}
LAST_RESULTS = {}

MAX_WAITS_PER_INST = 1  # this walrus rejects >=2 sync waits on an instruction
WAITS_PER_EVSEM = 2


def _hoist_excess_waits(bir: dict) -> dict:
    """Move sync waits of any instruction carrying more than
    MAX_WAITS_PER_INST onto EVSEM no-ops inserted right before it on the
    same engine queue (in-order execution preserves semantics)."""
    for func in bir.get("functions", []):
        for block in func.get("blocks", []):
            new_insts = []
            for inst in block.get("instructions", []):
                si = inst.get("sync_info") or {}
                waits = si.get("on_wait") or []
                if (
                    len(waits) > MAX_WAITS_PER_INST
                    and inst.get("opcode") != "EventSemaphore"
                ):
                    for k in range(0, len(waits), WAITS_PER_EVSEM):
                        chunk = waits[k : k + WAITS_PER_EVSEM]
                        nop = {
                            "engine": inst["engine"],
                            "ins": [],
                            "outs": [],
                            "name": f"{inst['name']}-hoist{k}",
                            "opcode": "EventSemaphore",
                            "sync_info": {
                                "on_update": [
                                    {
                                        "ant_name": chunk[0]["ant_name"],
                                        "id": chunk[0]["id"],
                                        "sync_type": "semaphore",
                                        "update_mode": "sem-add-imm",
                                        "update_value": 0,
                                    }
                                ],
                                "on_wait": chunk,
                            },
                        }
                        if "debug" in inst:
                            nop["debug"] = inst["debug"]
                        new_insts.append(nop)
                    si["on_wait"] = []
                new_insts.append(inst)
            block["instructions"] = new_insts
    return bir


class PatchedBass(bass.Bass):
    """See module docstring: skips the unsupported end-of-kernel semaphore
    RANGE_CLEAR/DMA_RESET instructions and hoists excess sync waits at
    serialization time."""

    def clear_and_free_semaphores(self, sems):
        if not sems:
            return
        sem_nums = [s.num if isinstance(s, SemaphoreHandle) else s for s in sems]
        self._state.prepend_free_semaphores(sem_nums)
        for poison_set in self._tile_sem_poison_stack:
            poison_set.update(sem_nums)

    def to_json_bytes(self):
        raw = super().to_json_bytes()
        bir = json.loads(raw)
        bir = _hoist_excess_waits(bir)
        return json.dumps(bir).encode()


class PatchedTC(tile.TileContext):
    """Move the terminal waits off the Drain (rejected by this walrus) onto
    chunked EVSEM no-ops on the sync queue; in-order execution then fences
    the wait-free Drain behind them."""

    def _drain_and_barrier(self, tick_clock, wait_clock):
        nc = self.nc
        gc = tick_clock.global_clock
        sems = list(self.sems.allocated().values())
        if sems:
            dummy = sems[0]
            procs = [p for p in range(N_PROCS) if gc[p] > 0]
            for i in range(0, len(procs), WAITS_PER_EVSEM):
                chunk = procs[i : i + WAITS_PER_EVSEM]
                part = VectorClock(
                    [gc[p] if p in chunk else 0 for p in range(N_PROCS)]
                )
                nop = nc.sync.sem_inc(dummy, 0)
                wait_clock.add_sem_waits(nop.ins, ScopedClock({None: part}))
        nc.sync.drain()
        popped = nc._tile_sem_poison_stack.pop()
        assert popped is self._sem_poison
        nc.clear_and_free_semaphores(list(self.sems.allocated().values()))


def _build_nc() -> bass.Bass:
    nc = PatchedBass(
        trn_type="TRN2",
        target_bir_lowering=False,
        debug=False,
        num_devices=NCORES,
    )
    enc = nc.dram_tensor("enc", [SHARD, HID], F32, kind="ExternalInput")
    vin = nc.dram_tensor("vin", [HID], F32, kind="ExternalInput")
    out_exp = nc.dram_tensor("exp_out", [SHARD], F32, kind="ExternalOutput")
    out_stats = nc.dram_tensor("stats_out", [P * 2], F32, kind="ExternalOutput")

    # s_local = p*TCOLS + t  ->  view enc as [p, t, h]
    enc_v = enc.ap().rearrange("(p t) h -> p t h", t=TCOLS)
    out_exp_v = out_exp.ap().rearrange("(p t) -> p t", t=TCOLS)
    out_stats_v = out_stats.ap().rearrange("(p two) -> p two", two=2)

    with PatchedTC(nc) as tc, ExitStack() as ctx:
        # Big tiles all stay resident (7 x 2MB + 4 x 512KB + scratch fits in
        # SBUF): no slot recycling, so no DMA is ever gated on compute.
        loads = ctx.enter_context(tc.tile_pool(name="loads", bufs=NDMA - 1))
        firsts = ctx.enter_context(tc.tile_pool(name="firsts", bufs=TPD))
        scratch = ctx.enter_context(tc.tile_pool(name="scratch", bufs=6))
        dead = ctx.enter_context(tc.tile_pool(name="dead", bufs=4))
        singles = ctx.enter_context(tc.tile_pool(name="singles", bufs=1))

        # v replicated across all 128 partitions (broadcast DMA from DRAM).
        # v_rep and the first columns ride the second HWDGE ring
        # (qActDynamicHW, via the scalar engine) so they land while the SP
        # ring is still streaming the big tiles — compute starts sooner.
        v_rep = singles.tile([P, HID], F32)
        v_src = bass.AP(tensor=vin.ap().tensor, offset=0, ap=[[0, P], [1, HID]])
        nc.scalar.dma_start(out=v_rep, in_=v_src)

        e_sbuf = singles.tile([P, TCOLS], F32)

        # ---- energies: e[p, t] = dot(enc[s=p*32+t, :], v) ----
        # The two full passes over the data (elementwise multiply, then
        # row reduction) are spread over three engines so none exceeds
        # the DMA streaming window: most multiplies on DVE (a few on
        # GPSIMD), most reductions on ACT via activation-Copy+accumulate
        # (a few on DVE).  The first 2 MB tile is loaded as four 512 KB
        # pieces so compute starts as soon as the first column lands.
        def do_col(col, col_ap):
            prod = scratch.tile([P, HID], F32)
            nc.vector.tensor_mul(prod, col_ap, v_rep)
            e_col = e_sbuf[:, col : col + 1]
            if col % 10 == 9:
                nc.vector.reduce_sum(e_col, prod, axis=mybir.AxisListType.X)
            else:
                sink = dead.tile([P, HID], F32)
                nc.scalar.activation(
                    out=sink,
                    in_=prod,
                    func=mybir.ActivationFunctionType.Copy,
                    accum_out=e_col,
                )

        for tt in range(TPD):
            first_tile = firsts.tile([P, 1, HID], F32, tag="first")
            nc.scalar.dma_start(out=first_tile, in_=enc_v[:, tt : tt + 1, :])
            do_col(tt, first_tile[:, 0, :])
        for j in range(1, NDMA):
            enc_tile = loads.tile([P, TPD, HID], F32)
            nc.sync.dma_start(
                out=enc_tile, in_=enc_v[:, j * TPD : (j + 1) * TPD, :]
            )
            for tt in range(TPD):
                do_col(j * TPD + tt, enc_tile[:, tt, :])

        # ---- per-partition softmax stats + exp ----
        stats = singles.tile([P, 2], F32)  # [:,0]=m_part  [:,1]=s_part
        neg_m = singles.tile([P, 1], F32)
        exp_pp = singles.tile([P, TCOLS], F32)
        nc.vector.reduce_max(stats[:, 0:1], e_sbuf, axis=mybir.AxisListType.X)
        nc.scalar.mul(neg_m, stats[:, 0:1], -1.0)
        nc.scalar.activation(
            out=exp_pp,
            in_=e_sbuf,
            func=mybir.ActivationFunctionType.Exp,
            bias=neg_m,
            scale=1.0,
            accum_out=stats[:, 1:2],
        )
        nc.sync.dma_start(out=out_exp_v, in_=exp_pp)
        nc.sync.dma_start(out=out_stats_v, in_=stats)

    return nc


_NC_CACHE = {}


def _get_nc() -> bass.Bass:
    if "nc" not in _NC_CACHE:
        _NC_CACHE["nc"] = _build_nc()
    return _NC_CACHE["nc"]


def kernel(hidden, encoder_outputs, W, b) -> np.ndarray:
    hidden = np.asarray(hidden, dtype=np.float32)
    encoder_outputs = np.ascontiguousarray(
        np.asarray(encoder_outputs, dtype=np.float32)
    )
    W = np.asarray(W, dtype=np.float32)

    # v = W.T @ h in f64 (tiny); b@h cancels in the softmax.
    h = hidden.reshape(-1).astype(np.float64)
    v = (W.astype(np.float64).T @ h).astype(np.float32)

    in_maps = [
        {
            "enc": np.ascontiguousarray(
                encoder_outputs[c * SHARD : (c + 1) * SHARD]
            ),
            "vin": v,
        }
        for c in range(NCORES)
    ]

    nc = _get_nc()
    res = run_bass_kernel_spmd(
        nc,
        in_maps,
        core_ids=list(range(NCORES)),
        trace=TRACE["on"],
    )
    LAST_RESULTS["res"] = res

    # ---- unshard + global softmax combine (tiny: 2*1024 stats floats) ----
    exp_pp = np.stack(
        [res.results[c]["exp_out"].reshape(P, TCOLS) for c in range(NCORES)]
    )  # [C, P, T] with s_global = c*SHARD + p*TCOLS + t
    stats = np.stack(
        [res.results[c]["stats_out"].reshape(P, 2) for c in range(NCORES)]
    )  # [C, P, 2]
    m = stats[:, :, 0].astype(np.float64)  # [C, P]
    s = stats[:, :, 1].astype(np.float64)
    gmax = m.max()
    gsum = float((s * np.exp(m - gmax)).sum())
    w = (np.exp(m - gmax) / gsum)[:, :, None]  # [C, P, 1]
    attn = (exp_pp.astype(np.float64) * w).astype(np.float32)
    return attn.reshape(1, 1, SEQ)


# revision 17
# speedup vs baseline: 1.2611x; 1.2611x over previous
"""Bass/Trainium2 kernel for nn_Attn_22814866276758.

Computation (reference):
    h = hidden[-1, 0]                            # [H]
    proj = enc @ W.T + b                         # [S, H]
    energies = proj @ h                          # [S]
    attn = softmax(energies)                     # [1, 1, S]

Algebraic collapse: energies = enc @ (W.T @ h) + (b @ h).  The constant
b @ h is uniform over S, so it cancels inside softmax.  The kernel is
therefore a memory-bound matvec over the 128 MB encoder_outputs plus a
global softmax.

Distribution (8 cores):
  - enc sharded over seq: each core owns [4096, 1024] (16 MB).
  - v = W.T @ h  (tiny) precomputed on host, replicated to all cores.
  - Each core: e[p, t] = dot(enc_row, v); the elementwise multiply runs
    on the DVE and the row reduction on the ACT engine (activation-Copy
    with accumulate), so the two passes overlap.  Local row index
    s = p*32 + t (p = SBUF partition).
  - Each core then computes per-partition online-softmax stats
    (m = row max, s = sum of exp(e-m)) and writes exp(e-m) plus the
    [128, 2] stats.  The global combine — max/sum over the 8*128 stats
    pairs and one scale per element — happens on the host during the
    unshard (an on-device all-gather of the same stats measured 23 us
    of RDH transfer + ~15 us of trigger latency for 8 KB, dwarfing the
    math it feeds).

Toolchain workarounds (this container's walrus build):
  - EVENT_SEMAPHORE_RANGE_CLEAR / DMA_QUEUE_RESET at Tile exit are
    rejected ("ISA wrong length") -> skipped (PatchedBass).
  - Sync waits on the terminal Drain are rejected ("Too many sync wait
    commands") -> moved onto EVSEM no-ops (PatchedTC).
  - Any instruction with >=2 sync waits is rejected -> waits hoisted
    onto EVSEM no-ops at BIR-JSON level (PatchedBass.to_json_bytes).
  - TensorTensorReduce opcode is unknown -> use mul + reduce instead.
"""

import json
from contextlib import ExitStack

import numpy as np

import concourse.bass as bass
import concourse.mybir as mybir
import concourse.tile as tile
from concourse.bass import SemaphoreHandle
from concourse.bass_utils import run_bass_kernel_spmd
from concourse.tile_sem_assignment import N_PROCS
from concourse.vector_clock import ScopedClock, VectorClock

SEQ = 32768
HID = 1024
NCORES = 8
SHARD = SEQ // NCORES  # 4096
P = 128  # SBUF partitions
TCOLS = SHARD // P  # 32 energy columns per core; s_local = p*TCOLS + t
TPD = 4  # seq-columns per DMA: tile = [128, TPD, 1024] = 2 MB
NDMA = TCOLS // TPD
F32 = mybir.dt.float32

# test.py pokes these to get a profiled run; harness path keeps defaults.
TRACE = {"on": False}
LAST_RESULTS = {}

MAX_WAITS_PER_INST = 1  # this walrus rejects >=2 sync waits on an instruction
WAITS_PER_EVSEM = 2


def _hoist_excess_waits(bir: dict) -> dict:
    """Move sync waits of any instruction carrying more than
    MAX_WAITS_PER_INST onto EVSEM no-ops inserted right before it on the
    same engine queue (in-order execution preserves semantics)."""
    for func in bir.get("functions", []):
        for block in func.get("blocks", []):
            new_insts = []
            for inst in block.get("instructions", []):
                si = inst.get("sync_info") or {}
                waits = si.get("on_wait") or []
                if (
                    len(waits) > MAX_WAITS_PER_INST
                    and inst.get("opcode") != "EventSemaphore"
                ):
                    for k in range(0, len(waits), WAITS_PER_EVSEM):
                        chunk = waits[k : k + WAITS_PER_EVSEM]
                        nop = {
                            "engine": inst["engine"],
                            "ins": [],
                            "outs": [],
                            "name": f"{inst['name']}-hoist{k}",
                            "opcode": "EventSemaphore",
                            "sync_info": {
                                "on_update": [
                                    {
                                        "ant_name": chunk[0]["ant_name"],
                                        "id": chunk[0]["id"],
                                        "sync_type": "semaphore",
                                        "update_mode": "sem-add-imm",
                                        "update_value": 0,
                                    }
                                ],
                                "on_wait": chunk,
                            },
                        }
                        if "debug" in inst:
                            nop["debug"] = inst["debug"]
                        new_insts.append(nop)
                    si["on_wait"] = []
                new_insts.append(inst)
            block["instructions"] = new_insts
    return bir


class PatchedBass(bass.Bass):
    """See module docstring: skips the unsupported end-of-kernel semaphore
    RANGE_CLEAR/DMA_RESET instructions and hoists excess sync waits at
    serialization time."""

    def clear_and_free_semaphores(self, sems):
        if not sems:
            return
        sem_nums = [s.num if isinstance(s, SemaphoreHandle) else s for s in sems]
        self._state.prepend_free_semaphores(sem_nums)
        for poison_set in self._tile_sem_poison_stack:
            poison_set.update(sem_nums)

    def to_json_bytes(self):
        raw = super().to_json_bytes()
        bir = json.loads(raw)
        bir = _hoist_excess_waits(bir)
        return json.dumps(bir).encode()


class PatchedTC(tile.TileContext):
    """Move the terminal waits off the Drain (rejected by this walrus) onto
    chunked EVSEM no-ops on the sync queue; in-order execution then fences
    the wait-free Drain behind them."""

    def _drain_and_barrier(self, tick_clock, wait_clock):
        nc = self.nc
        gc = tick_clock.global_clock
        sems = list(self.sems.allocated().values())
        if sems:
            dummy = sems[0]
            procs = [p for p in range(N_PROCS) if gc[p] > 0]
            for i in range(0, len(procs), WAITS_PER_EVSEM):
                chunk = procs[i : i + WAITS_PER_EVSEM]
                part = VectorClock(
                    [gc[p] if p in chunk else 0 for p in range(N_PROCS)]
                )
                nop = nc.sync.sem_inc(dummy, 0)
                wait_clock.add_sem_waits(nop.ins, ScopedClock({None: part}))
        nc.sync.drain()
        popped = nc._tile_sem_poison_stack.pop()
        assert popped is self._sem_poison
        nc.clear_and_free_semaphores(list(self.sems.allocated().values()))


def _build_nc() -> bass.Bass:
    nc = PatchedBass(
        trn_type="TRN2",
        target_bir_lowering=False,
        debug=False,
        num_devices=NCORES,
    )
    enc = nc.dram_tensor("enc", [SHARD, HID], F32, kind="ExternalInput")
    vin = nc.dram_tensor("vin", [P, HID], F32, kind="ExternalInput")
    out_exp = nc.dram_tensor("exp_out", [SHARD], F32, kind="ExternalOutput")
    out_stats = nc.dram_tensor("stats_out", [P * 2], F32, kind="ExternalOutput")

    # s_local = p*TCOLS + t  ->  view enc as [p, t, h]
    enc_v = enc.ap().rearrange("(p t) h -> p t h", t=TCOLS)
    out_exp_v = out_exp.ap().rearrange("(p t) -> p t", t=TCOLS)
    out_stats_v = out_stats.ap().rearrange("(p two) -> p two", two=2)

    with PatchedTC(nc) as tc, ExitStack() as ctx:
        # Big tiles all stay resident (7 x 2MB + 4 x 512KB + scratch fits in
        # SBUF): no slot recycling, so no DMA is ever gated on compute.
        loads = ctx.enter_context(tc.tile_pool(name="loads", bufs=NDMA - 1))
        firsts = ctx.enter_context(tc.tile_pool(name="firsts", bufs=TPD))
        scratch = ctx.enter_context(tc.tile_pool(name="scratch", bufs=6))
        dead = ctx.enter_context(tc.tile_pool(name="dead", bufs=4))
        singles = ctx.enter_context(tc.tile_pool(name="singles", bufs=1))

        # v arrives pre-replicated from the host as [128, 1024] — a plain
        # contiguous 512 KB load at the head of the DMA FIFO.  (A stride-0
        # broadcast DMA from DRAM measured ~4 us here.)
        v_rep = singles.tile([P, HID], F32)
        nc.sync.dma_start(out=v_rep, in_=vin.ap())

        e_sbuf = singles.tile([P, TCOLS], F32)

        # ---- energies: e[p, t] = dot(enc[s=p*32+t, :], v) ----
        # The two full passes over the data (elementwise multiply, then
        # row reduction) are spread over three engines so none exceeds
        # the DMA streaming window: most multiplies on DVE (a few on
        # GPSIMD), most reductions on ACT via activation-Copy+accumulate
        # (a few on DVE).  The first 2 MB tile is loaded as four 512 KB
        # pieces so compute starts as soon as the first column lands.
        def do_col(col, col_ap):
            prod = scratch.tile([P, HID], F32)
            nc.vector.tensor_mul(prod, col_ap, v_rep)
            e_col = e_sbuf[:, col : col + 1]
            if col % 10 == 9:
                nc.vector.reduce_sum(e_col, prod, axis=mybir.AxisListType.X)
            else:
                sink = dead.tile([P, HID], F32)
                nc.scalar.activation(
                    out=sink,
                    in_=prod,
                    func=mybir.ActivationFunctionType.Copy,
                    accum_out=e_col,
                )

        for tt in range(TPD):
            first_tile = firsts.tile([P, 1, HID], F32, tag="first")
            nc.sync.dma_start(out=first_tile, in_=enc_v[:, tt : tt + 1, :])
            do_col(tt, first_tile[:, 0, :])
        for j in range(1, NDMA):
            enc_tile = loads.tile([P, TPD, HID], F32)
            nc.sync.dma_start(
                out=enc_tile, in_=enc_v[:, j * TPD : (j + 1) * TPD, :]
            )
            for tt in range(TPD):
                do_col(j * TPD + tt, enc_tile[:, tt, :])

        # ---- per-partition softmax stats + exp ----
        stats = singles.tile([P, 2], F32)  # [:,0]=m_part  [:,1]=s_part
        neg_m = singles.tile([P, 1], F32)
        exp_pp = singles.tile([P, TCOLS], F32)
        nc.vector.reduce_max(stats[:, 0:1], e_sbuf, axis=mybir.AxisListType.X)
        nc.scalar.mul(neg_m, stats[:, 0:1], -1.0)
        nc.scalar.activation(
            out=exp_pp,
            in_=e_sbuf,
            func=mybir.ActivationFunctionType.Exp,
            bias=neg_m,
            scale=1.0,
            accum_out=stats[:, 1:2],
        )
        nc.sync.dma_start(out=out_exp_v, in_=exp_pp)
        nc.sync.dma_start(out=out_stats_v, in_=stats)

    return nc


_NC_CACHE = {}


def _get_nc() -> bass.Bass:
    if "nc" not in _NC_CACHE:
        _NC_CACHE["nc"] = _build_nc()
    return _NC_CACHE["nc"]


def kernel(hidden, encoder_outputs, W, b) -> np.ndarray:
    hidden = np.asarray(hidden, dtype=np.float32)
    encoder_outputs = np.ascontiguousarray(
        np.asarray(encoder_outputs, dtype=np.float32)
    )
    W = np.asarray(W, dtype=np.float32)

    # v = W.T @ h in f64 (tiny); b@h cancels in the softmax.
    h = hidden.reshape(-1).astype(np.float64)
    v = (W.astype(np.float64).T @ h).astype(np.float32)
    v_rep_host = np.ascontiguousarray(np.broadcast_to(v, (P, HID)))

    in_maps = [
        {
            "enc": np.ascontiguousarray(
                encoder_outputs[c * SHARD : (c + 1) * SHARD]
            ),
            "vin": v_rep_host,
        }
        for c in range(NCORES)
    ]

    nc = _get_nc()
    res = run_bass_kernel_spmd(
        nc,
        in_maps,
        core_ids=list(range(NCORES)),
        trace=TRACE["on"],
    )
    LAST_RESULTS["res"] = res

    # ---- unshard + global softmax combine (tiny: 2*1024 stats floats) ----
    exp_pp = np.stack(
        [res.results[c]["exp_out"].reshape(P, TCOLS) for c in range(NCORES)]
    )  # [C, P, T] with s_global = c*SHARD + p*TCOLS + t
    stats = np.stack(
        [res.results[c]["stats_out"].reshape(P, 2) for c in range(NCORES)]
    )  # [C, P, 2]
    m = stats[:, :, 0].astype(np.float64)  # [C, P]
    s = stats[:, :, 1].astype(np.float64)
    gmax = m.max()
    gsum = float((s * np.exp(m - gmax)).sum())
    w = (np.exp(m - gmax) / gsum)[:, :, None]  # [C, P, 1]
    attn = (exp_pp.astype(np.float64) * w).astype(np.float32)
    return attn.reshape(1, 1, SEQ)


# revision 19
# speedup vs baseline: 1.3987x; 1.1091x over previous
"""Bass/Trainium2 kernel for nn_Attn_22814866276758.

Computation (reference):
    h = hidden[-1, 0]                            # [H]
    proj = enc @ W.T + b                         # [S, H]
    energies = proj @ h                          # [S]
    attn = softmax(energies)                     # [1, 1, S]

Algebraic collapse: energies = enc @ (W.T @ h) + (b @ h).  The constant
b @ h is uniform over S, so it cancels inside softmax.  The kernel is
therefore a memory-bound matvec over the 128 MB encoder_outputs plus a
global softmax.

Distribution (8 cores):
  - enc sharded over seq: each core owns [4096, 1024] (16 MB).
  - v = W.T @ h  (tiny) precomputed on host, replicated to all cores.
  - Each core: e[p, t] = dot(enc_row, v); the elementwise multiply runs
    on the DVE and the row reduction on the ACT engine (activation-Copy
    with accumulate), so the two passes overlap.  Local row index
    s = p*32 + t (p = SBUF partition).
  - Each core then computes per-partition online-softmax stats
    (m = row max, s = sum of exp(e-m)) and writes exp(e-m) plus the
    [128, 2] stats.  The global combine — max/sum over the 8*128 stats
    pairs and one scale per element — happens on the host during the
    unshard (an on-device all-gather of the same stats measured 23 us
    of RDH transfer + ~15 us of trigger latency for 8 KB, dwarfing the
    math it feeds).

Toolchain workarounds (this container's walrus build):
  - EVENT_SEMAPHORE_RANGE_CLEAR / DMA_QUEUE_RESET at Tile exit are
    rejected ("ISA wrong length") -> skipped (PatchedBass).
  - Sync waits on the terminal Drain are rejected ("Too many sync wait
    commands") -> moved onto EVSEM no-ops (PatchedTC).
  - Any instruction with >=2 sync waits is rejected -> waits hoisted
    onto EVSEM no-ops at BIR-JSON level (PatchedBass.to_json_bytes).
  - TensorTensorReduce opcode is unknown -> use mul + reduce instead.
"""

import json
from contextlib import ExitStack

import numpy as np

import concourse.bass as bass
import concourse.mybir as mybir
import concourse.tile as tile
from concourse.bass import SemaphoreHandle
from concourse.bass_utils import run_bass_kernel_spmd
from concourse.tile_sem_assignment import N_PROCS
from concourse.vector_clock import ScopedClock, VectorClock

SEQ = 32768
HID = 1024
NCORES = 8
SHARD = SEQ // NCORES  # 4096
P = 128  # SBUF partitions
TCOLS = SHARD // P  # 32 energy columns per core; s_local = p*TCOLS + t
TPD = 4  # seq-columns per DMA: tile = [128, TPD, 1024] = 2 MB
NDMA = TCOLS // TPD
F32 = mybir.dt.float32

# test.py pokes these to get a profiled run; harness path keeps defaults.
TRACE = {"on": False}
LAST_RESULTS = {}

MAX_WAITS_PER_INST = 1  # this walrus rejects >=2 sync waits on an instruction
WAITS_PER_EVSEM = 2


def _hoist_excess_waits(bir: dict) -> dict:
    """Move sync waits of any instruction carrying more than
    MAX_WAITS_PER_INST onto EVSEM no-ops inserted right before it on the
    same engine queue (in-order execution preserves semantics)."""
    for func in bir.get("functions", []):
        for block in func.get("blocks", []):
            new_insts = []
            for inst in block.get("instructions", []):
                si = inst.get("sync_info") or {}
                waits = si.get("on_wait") or []
                if (
                    len(waits) > MAX_WAITS_PER_INST
                    and inst.get("opcode") != "EventSemaphore"
                ):
                    for k in range(0, len(waits), WAITS_PER_EVSEM):
                        chunk = waits[k : k + WAITS_PER_EVSEM]
                        nop = {
                            "engine": inst["engine"],
                            "ins": [],
                            "outs": [],
                            "name": f"{inst['name']}-hoist{k}",
                            "opcode": "EventSemaphore",
                            "sync_info": {
                                "on_update": [
                                    {
                                        "ant_name": chunk[0]["ant_name"],
                                        "id": chunk[0]["id"],
                                        "sync_type": "semaphore",
                                        "update_mode": "sem-add-imm",
                                        "update_value": 0,
                                    }
                                ],
                                "on_wait": chunk,
                            },
                        }
                        if "debug" in inst:
                            nop["debug"] = inst["debug"]
                        new_insts.append(nop)
                    si["on_wait"] = []
                new_insts.append(inst)
            block["instructions"] = new_insts
    return bir


class PatchedBass(bass.Bass):
    """See module docstring: skips the unsupported end-of-kernel semaphore
    RANGE_CLEAR/DMA_RESET instructions and hoists excess sync waits at
    serialization time."""

    def clear_and_free_semaphores(self, sems):
        if not sems:
            return
        sem_nums = [s.num if isinstance(s, SemaphoreHandle) else s for s in sems]
        self._state.prepend_free_semaphores(sem_nums)
        for poison_set in self._tile_sem_poison_stack:
            poison_set.update(sem_nums)

    def to_json_bytes(self):
        raw = super().to_json_bytes()
        bir = json.loads(raw)
        bir = _hoist_excess_waits(bir)
        return json.dumps(bir).encode()


class PatchedTC(tile.TileContext):
    """Move the terminal waits off the Drain (rejected by this walrus) onto
    chunked EVSEM no-ops on the sync queue; in-order execution then fences
    the wait-free Drain behind them."""

    def _drain_and_barrier(self, tick_clock, wait_clock):
        nc = self.nc
        gc = tick_clock.global_clock
        sems = list(self.sems.allocated().values())
        if sems:
            dummy = sems[0]
            procs = [p for p in range(N_PROCS) if gc[p] > 0]
            for i in range(0, len(procs), WAITS_PER_EVSEM):
                chunk = procs[i : i + WAITS_PER_EVSEM]
                part = VectorClock(
                    [gc[p] if p in chunk else 0 for p in range(N_PROCS)]
                )
                nop = nc.sync.sem_inc(dummy, 0)
                wait_clock.add_sem_waits(nop.ins, ScopedClock({None: part}))
        nc.sync.drain()
        popped = nc._tile_sem_poison_stack.pop()
        assert popped is self._sem_poison
        nc.clear_and_free_semaphores(list(self.sems.allocated().values()))


def _build_nc() -> bass.Bass:
    nc = PatchedBass(
        trn_type="TRN2",
        target_bir_lowering=False,
        debug=False,
        num_devices=NCORES,
    )
    enc = nc.dram_tensor("enc", [SHARD, HID], F32, kind="ExternalInput")
    vin = nc.dram_tensor("vin", [P, HID], F32, kind="ExternalInput")
    out_exp = nc.dram_tensor("exp_out", [SHARD], F32, kind="ExternalOutput")
    out_stats = nc.dram_tensor("stats_out", [P * 2], F32, kind="ExternalOutput")

    # s_local = p*TCOLS + t  ->  view enc as [p, t, h]
    enc_v = enc.ap().rearrange("(p t) h -> p t h", t=TCOLS)
    out_exp_v = out_exp.ap().rearrange("(p t) -> p t", t=TCOLS)
    out_stats_v = out_stats.ap().rearrange("(p two) -> p two", two=2)

    with PatchedTC(nc) as tc, ExitStack() as ctx:
        # Big tiles all stay resident (7 x 2MB + 4 x 512KB + scratch fits in
        # SBUF): no slot recycling, so no DMA is ever gated on compute.
        loads = ctx.enter_context(tc.tile_pool(name="loads", bufs=NDMA - 2))
        firsts = ctx.enter_context(tc.tile_pool(name="firsts", bufs=8))
        scratch = ctx.enter_context(tc.tile_pool(name="scratch", bufs=6))
        dead = ctx.enter_context(tc.tile_pool(name="dead", bufs=4))
        singles = ctx.enter_context(tc.tile_pool(name="singles", bufs=1))

        # v arrives pre-replicated from the host as [128, 1024] — a plain
        # contiguous 512 KB load at the head of the DMA FIFO.  (A stride-0
        # broadcast DMA from DRAM measured ~4 us here.)
        v_rep = singles.tile([P, HID], F32)
        nc.sync.dma_start(out=v_rep, in_=vin.ap())

        e_sbuf = singles.tile([P, TCOLS], F32)

        # ---- energies: e[p, t] = dot(enc[s=p*32+t, :], v) ----
        # The two full passes over the data (elementwise multiply, then
        # row reduction) are spread over three engines so none exceeds
        # the DMA streaming window: most multiplies on DVE (a few on
        # GPSIMD), most reductions on ACT via activation-Copy+accumulate
        # (a few on DVE).  The first 2 MB tile is loaded as four 512 KB
        # pieces so compute starts as soon as the first column lands.
        def do_col(col, col_ap):
            prod = scratch.tile([P, HID], F32)
            nc.vector.tensor_mul(prod, col_ap, v_rep)
            e_col = e_sbuf[:, col : col + 1]
            # a few reductions go to the DVE early on (it idles during the
            # DMA ramp); ACT takes the rest so the steady state is DVE-
            # mul-bound, never blocked at the end.
            if col in (3, 7, 11):
                nc.vector.reduce_sum(e_col, prod, axis=mybir.AxisListType.X)
            else:
                sink = dead.tile([P, HID], F32)
                nc.scalar.activation(
                    out=sink,
                    in_=prod,
                    func=mybir.ActivationFunctionType.Copy,
                    accum_out=e_col,
                )

        # smooth ramp: the first NFIRST columns arrive as single-column
        # 512 KB loads, the rest as 2 MB tiles — all on the in-order
        # HWDGE FIFO, issued up front (every tile stays resident).
        NFIRST = 8
        for tt in range(NFIRST):
            first_tile = firsts.tile([P, 1, HID], F32, tag="first")
            nc.sync.dma_start(out=first_tile, in_=enc_v[:, tt : tt + 1, :])
            do_col(tt, first_tile[:, 0, :])
        for j in range(NFIRST // TPD, NDMA):
            enc_tile = loads.tile([P, TPD, HID], F32)
            nc.sync.dma_start(
                out=enc_tile, in_=enc_v[:, j * TPD : (j + 1) * TPD, :]
            )
            for tt in range(TPD):
                do_col(j * TPD + tt, enc_tile[:, tt, :])

        # ---- per-partition softmax stats + exp ----
        stats = singles.tile([P, 2], F32)  # [:,0]=m_part  [:,1]=s_part
        neg_m = singles.tile([P, 1], F32)
        exp_pp = singles.tile([P, TCOLS], F32)
        nc.vector.reduce_max(stats[:, 0:1], e_sbuf, axis=mybir.AxisListType.X)
        nc.scalar.mul(neg_m, stats[:, 0:1], -1.0)
        nc.scalar.activation(
            out=exp_pp,
            in_=e_sbuf,
            func=mybir.ActivationFunctionType.Exp,
            bias=neg_m,
            scale=1.0,
            accum_out=stats[:, 1:2],
        )
        nc.sync.dma_start(out=out_exp_v, in_=exp_pp)
        nc.sync.dma_start(out=out_stats_v, in_=stats)

    return nc


_NC_CACHE = {}


def _get_nc() -> bass.Bass:
    if "nc" not in _NC_CACHE:
        _NC_CACHE["nc"] = _build_nc()
    return _NC_CACHE["nc"]


def kernel(hidden, encoder_outputs, W, b) -> np.ndarray:
    hidden = np.asarray(hidden, dtype=np.float32)
    encoder_outputs = np.ascontiguousarray(
        np.asarray(encoder_outputs, dtype=np.float32)
    )
    W = np.asarray(W, dtype=np.float32)

    # v = W.T @ h in f64 (tiny); b@h cancels in the softmax.
    h = hidden.reshape(-1).astype(np.float64)
    v = (W.astype(np.float64).T @ h).astype(np.float32)
    v_rep_host = np.ascontiguousarray(np.broadcast_to(v, (P, HID)))

    in_maps = [
        {
            "enc": np.ascontiguousarray(
                encoder_outputs[c * SHARD : (c + 1) * SHARD]
            ),
            "vin": v_rep_host,
        }
        for c in range(NCORES)
    ]

    nc = _get_nc()
    res = run_bass_kernel_spmd(
        nc,
        in_maps,
        core_ids=list(range(NCORES)),
        trace=TRACE["on"],
    )
    LAST_RESULTS["res"] = res

    # ---- unshard + global softmax combine (tiny: 2*1024 stats floats) ----
    exp_pp = np.stack(
        [res.results[c]["exp_out"].reshape(P, TCOLS) for c in range(NCORES)]
    )  # [C, P, T] with s_global = c*SHARD + p*TCOLS + t
    stats = np.stack(
        [res.results[c]["stats_out"].reshape(P, 2) for c in range(NCORES)]
    )  # [C, P, 2]
    m = stats[:, :, 0].astype(np.float64)  # [C, P]
    s = stats[:, :, 1].astype(np.float64)
    gmax = m.max()
    gsum = float((s * np.exp(m - gmax)).sum())
    w = (np.exp(m - gmax) / gsum)[:, :, None]  # [C, P, 1]
    attn = (exp_pp.astype(np.float64) * w).astype(np.float32)
    return attn.reshape(1, 1, SEQ)


# revision 20
# speedup vs baseline: 1.4046x; 1.0042x over previous
"""Bass/Trainium2 kernel for nn_Attn_22814866276758.

Computation (reference):
    h = hidden[-1, 0]                            # [H]
    proj = enc @ W.T + b                         # [S, H]
    energies = proj @ h                          # [S]
    attn = softmax(energies)                     # [1, 1, S]

Algebraic collapse: energies = enc @ (W.T @ h) + (b @ h).  The constant
b @ h is uniform over S, so it cancels inside softmax.  The kernel is
therefore a memory-bound matvec over the 128 MB encoder_outputs plus a
global softmax.

Distribution (8 cores):
  - enc sharded over seq: each core owns [4096, 1024] (16 MB).
  - v = W.T @ h  (tiny) precomputed on host, replicated to all cores.
  - Each core: e[p, t] = dot(enc_row, v); the elementwise multiply runs
    on the DVE and the row reduction on the ACT engine (activation-Copy
    with accumulate), so the two passes overlap.  Local row index
    s = p*32 + t (p = SBUF partition).
  - Each core then computes per-partition online-softmax stats
    (m = row max, s = sum of exp(e-m)) and writes exp(e-m) plus the
    [128, 2] stats.  The global combine — max/sum over the 8*128 stats
    pairs and one scale per element — happens on the host during the
    unshard (an on-device all-gather of the same stats measured 23 us
    of RDH transfer + ~15 us of trigger latency for 8 KB, dwarfing the
    math it feeds).

Toolchain workarounds (this container's walrus build):
  - EVENT_SEMAPHORE_RANGE_CLEAR / DMA_QUEUE_RESET at Tile exit are
    rejected ("ISA wrong length") -> skipped (PatchedBass).
  - Sync waits on the terminal Drain are rejected ("Too many sync wait
    commands") -> moved onto EVSEM no-ops (PatchedTC).
  - Any instruction with >=2 sync waits is rejected -> waits hoisted
    onto EVSEM no-ops at BIR-JSON level (PatchedBass.to_json_bytes).
  - TensorTensorReduce opcode is unknown -> use mul + reduce instead.
"""

import json
from contextlib import ExitStack

import numpy as np

import concourse.bass as bass
import concourse.mybir as mybir
import concourse.tile as tile
from concourse.bass import SemaphoreHandle
from concourse.bass_utils import run_bass_kernel_spmd
from concourse.tile_sem_assignment import N_PROCS
from concourse.vector_clock import ScopedClock, VectorClock

SEQ = 32768
HID = 1024
NCORES = 8
SHARD = SEQ // NCORES  # 4096
P = 128  # SBUF partitions
TCOLS = SHARD // P  # 32 energy columns per core; s_local = p*TCOLS + t
TPD = 4  # seq-columns per DMA: tile = [128, TPD, 1024] = 2 MB
NDMA = TCOLS // TPD
F32 = mybir.dt.float32

# test.py pokes these to get a profiled run; harness path keeps defaults.
TRACE = {"on": False}
LAST_RESULTS = {}

MAX_WAITS_PER_INST = 1  # this walrus rejects >=2 sync waits on an instruction
WAITS_PER_EVSEM = 2


def _hoist_excess_waits(bir: dict) -> dict:
    """Move sync waits of any instruction carrying more than
    MAX_WAITS_PER_INST onto EVSEM no-ops inserted right before it on the
    same engine queue (in-order execution preserves semantics)."""
    for func in bir.get("functions", []):
        for block in func.get("blocks", []):
            new_insts = []
            for inst in block.get("instructions", []):
                si = inst.get("sync_info") or {}
                waits = si.get("on_wait") or []
                if (
                    len(waits) > MAX_WAITS_PER_INST
                    and inst.get("opcode") != "EventSemaphore"
                ):
                    for k in range(0, len(waits), WAITS_PER_EVSEM):
                        chunk = waits[k : k + WAITS_PER_EVSEM]
                        nop = {
                            "engine": inst["engine"],
                            "ins": [],
                            "outs": [],
                            "name": f"{inst['name']}-hoist{k}",
                            "opcode": "EventSemaphore",
                            "sync_info": {
                                "on_update": [
                                    {
                                        "ant_name": chunk[0]["ant_name"],
                                        "id": chunk[0]["id"],
                                        "sync_type": "semaphore",
                                        "update_mode": "sem-add-imm",
                                        "update_value": 0,
                                    }
                                ],
                                "on_wait": chunk,
                            },
                        }
                        if "debug" in inst:
                            nop["debug"] = inst["debug"]
                        new_insts.append(nop)
                    si["on_wait"] = []
                new_insts.append(inst)
            block["instructions"] = new_insts
    return bir


class PatchedBass(bass.Bass):
    """See module docstring: skips the unsupported end-of-kernel semaphore
    RANGE_CLEAR/DMA_RESET instructions and hoists excess sync waits at
    serialization time."""

    def clear_and_free_semaphores(self, sems):
        if not sems:
            return
        sem_nums = [s.num if isinstance(s, SemaphoreHandle) else s for s in sems]
        self._state.prepend_free_semaphores(sem_nums)
        for poison_set in self._tile_sem_poison_stack:
            poison_set.update(sem_nums)

    def to_json_bytes(self):
        raw = super().to_json_bytes()
        bir = json.loads(raw)
        bir = _hoist_excess_waits(bir)
        return json.dumps(bir).encode()


class PatchedTC(tile.TileContext):
    """Move the terminal waits off the Drain (rejected by this walrus) onto
    chunked EVSEM no-ops on the sync queue; in-order execution then fences
    the wait-free Drain behind them."""

    def _drain_and_barrier(self, tick_clock, wait_clock):
        nc = self.nc
        gc = tick_clock.global_clock
        sems = list(self.sems.allocated().values())
        if sems:
            dummy = sems[0]
            procs = [p for p in range(N_PROCS) if gc[p] > 0]
            for i in range(0, len(procs), WAITS_PER_EVSEM):
                chunk = procs[i : i + WAITS_PER_EVSEM]
                part = VectorClock(
                    [gc[p] if p in chunk else 0 for p in range(N_PROCS)]
                )
                nop = nc.sync.sem_inc(dummy, 0)
                wait_clock.add_sem_waits(nop.ins, ScopedClock({None: part}))
        nc.sync.drain()
        popped = nc._tile_sem_poison_stack.pop()
        assert popped is self._sem_poison
        nc.clear_and_free_semaphores(list(self.sems.allocated().values()))


def _build_nc() -> bass.Bass:
    nc = PatchedBass(
        trn_type="TRN2",
        target_bir_lowering=False,
        debug=False,
        num_devices=NCORES,
    )
    enc = nc.dram_tensor("enc", [SHARD, HID], F32, kind="ExternalInput")
    vin = nc.dram_tensor("vin", [P, HID], F32, kind="ExternalInput")
    out_exp = nc.dram_tensor("exp_out", [SHARD], F32, kind="ExternalOutput")
    out_stats = nc.dram_tensor("stats_out", [P * 2], F32, kind="ExternalOutput")

    # s_local = p*TCOLS + t  ->  view enc as [p, t, h]
    enc_v = enc.ap().rearrange("(p t) h -> p t h", t=TCOLS)
    out_exp_v = out_exp.ap().rearrange("(p t) -> p t", t=TCOLS)
    out_stats_v = out_stats.ap().rearrange("(p two) -> p two", two=2)

    with PatchedTC(nc) as tc, ExitStack() as ctx:
        # Big tiles all stay resident (7 x 2MB + 4 x 512KB + scratch fits in
        # SBUF): no slot recycling, so no DMA is ever gated on compute.
        loads = ctx.enter_context(tc.tile_pool(name="loads", bufs=NDMA - 2))
        firsts = ctx.enter_context(tc.tile_pool(name="firsts", bufs=TCOLS))
        scratch = ctx.enter_context(tc.tile_pool(name="scratch", bufs=6))
        dead = ctx.enter_context(tc.tile_pool(name="dead", bufs=4))
        singles = ctx.enter_context(tc.tile_pool(name="singles", bufs=1))

        # v arrives pre-replicated from the host as [128, 1024] — a plain
        # contiguous 512 KB load at the head of the DMA FIFO.  (A stride-0
        # broadcast DMA from DRAM measured ~4 us here.)
        v_rep = singles.tile([P, HID], F32)
        nc.sync.dma_start(out=v_rep, in_=vin.ap())

        e_sbuf = singles.tile([P, TCOLS], F32)

        # ---- energies: e[p, t] = dot(enc[s=p*32+t, :], v) ----
        # The two full passes over the data (elementwise multiply, then
        # row reduction) are spread over three engines so none exceeds
        # the DMA streaming window: most multiplies on DVE (a few on
        # GPSIMD), most reductions on ACT via activation-Copy+accumulate
        # (a few on DVE).  The first 2 MB tile is loaded as four 512 KB
        # pieces so compute starts as soon as the first column lands.
        def do_col(col, col_ap):
            prod = scratch.tile([P, HID], F32)
            nc.vector.tensor_mul(prod, col_ap, v_rep)
            e_col = e_sbuf[:, col : col + 1]
            # a few reductions go to the DVE early on (it idles during the
            # DMA ramp); ACT takes the rest so the steady state is DVE-
            # mul-bound, never blocked at the end.
            if col in (3, 7, 11):
                nc.vector.reduce_sum(e_col, prod, axis=mybir.AxisListType.X)
            else:
                sink = dead.tile([P, HID], F32)
                nc.scalar.activation(
                    out=sink,
                    in_=prod,
                    func=mybir.ActivationFunctionType.Copy,
                    accum_out=e_col,
                )

        # uniform single-column 512 KB loads on the in-order HWDGE FIFO,
        # all issued up front (every tile stays resident): arrival
        # granularity matches the per-column compute, so the pipeline
        # rides the stream with no transition stalls.
        for col in range(TCOLS):
            col_tile = firsts.tile([P, 1, HID], F32, tag="first")
            nc.sync.dma_start(out=col_tile, in_=enc_v[:, col : col + 1, :])
            do_col(col, col_tile[:, 0, :])

        # ---- per-partition softmax stats + exp ----
        stats = singles.tile([P, 2], F32)  # [:,0]=m_part  [:,1]=s_part
        neg_m = singles.tile([P, 1], F32)
        exp_pp = singles.tile([P, TCOLS], F32)
        nc.vector.reduce_max(stats[:, 0:1], e_sbuf, axis=mybir.AxisListType.X)
        nc.scalar.mul(neg_m, stats[:, 0:1], -1.0)
        nc.scalar.activation(
            out=exp_pp,
            in_=e_sbuf,
            func=mybir.ActivationFunctionType.Exp,
            bias=neg_m,
            scale=1.0,
            accum_out=stats[:, 1:2],
        )
        nc.sync.dma_start(out=out_exp_v, in_=exp_pp)
        nc.sync.dma_start(out=out_stats_v, in_=stats)

    return nc


_NC_CACHE = {}


def _get_nc() -> bass.Bass:
    if "nc" not in _NC_CACHE:
        _NC_CACHE["nc"] = _build_nc()
    return _NC_CACHE["nc"]


def kernel(hidden, encoder_outputs, W, b) -> np.ndarray:
    hidden = np.asarray(hidden, dtype=np.float32)
    encoder_outputs = np.ascontiguousarray(
        np.asarray(encoder_outputs, dtype=np.float32)
    )
    W = np.asarray(W, dtype=np.float32)

    # v = W.T @ h in f64 (tiny); b@h cancels in the softmax.
    h = hidden.reshape(-1).astype(np.float64)
    v = (W.astype(np.float64).T @ h).astype(np.float32)
    v_rep_host = np.ascontiguousarray(np.broadcast_to(v, (P, HID)))

    in_maps = [
        {
            "enc": np.ascontiguousarray(
                encoder_outputs[c * SHARD : (c + 1) * SHARD]
            ),
            "vin": v_rep_host,
        }
        for c in range(NCORES)
    ]

    nc = _get_nc()
    res = run_bass_kernel_spmd(
        nc,
        in_maps,
        core_ids=list(range(NCORES)),
        trace=TRACE["on"],
    )
    LAST_RESULTS["res"] = res

    # ---- unshard + global softmax combine (tiny: 2*1024 stats floats) ----
    exp_pp = np.stack(
        [res.results[c]["exp_out"].reshape(P, TCOLS) for c in range(NCORES)]
    )  # [C, P, T] with s_global = c*SHARD + p*TCOLS + t
    stats = np.stack(
        [res.results[c]["stats_out"].reshape(P, 2) for c in range(NCORES)]
    )  # [C, P, 2]
    m = stats[:, :, 0].astype(np.float64)  # [C, P]
    s = stats[:, :, 1].astype(np.float64)
    gmax = m.max()
    gsum = float((s * np.exp(m - gmax)).sum())
    w = (np.exp(m - gmax) / gsum)[:, :, None]  # [C, P, 1]
    attn = (exp_pp.astype(np.float64) * w).astype(np.float32)
    return attn.reshape(1, 1, SEQ)


# revision 22
# speedup vs baseline: 1.4433x; 1.0276x over previous
"""Bass/Trainium2 kernel for nn_Attn_22814866276758.

Computation (reference):
    h = hidden[-1, 0]                            # [H]
    proj = enc @ W.T + b                         # [S, H]
    energies = proj @ h                          # [S]
    attn = softmax(energies)                     # [1, 1, S]

Algebraic collapse: energies = enc @ (W.T @ h) + (b @ h).  The constant
b @ h is uniform over S, so it cancels inside softmax.  The kernel is
therefore a memory-bound matvec over the 128 MB encoder_outputs plus a
global softmax.

Distribution (8 cores):
  - enc sharded over seq: each core owns [4096, 1024] (16 MB).
  - v = W.T @ h  (tiny) precomputed on host, replicated to all cores.
  - Each core: e[p, t] = dot(enc_row, v); the elementwise multiply runs
    on the DVE and the row reduction on the ACT engine (activation-Copy
    with accumulate), so the two passes overlap.  Local row index
    s = p*32 + t (p = SBUF partition).
  - Each core then computes per-partition online-softmax stats
    (m = row max, s = sum of exp(e-m)) and writes exp(e-m) plus the
    [128, 2] stats.  The global combine — max/sum over the 8*128 stats
    pairs and one scale per element — happens on the host during the
    unshard (an on-device all-gather of the same stats measured 23 us
    of RDH transfer + ~15 us of trigger latency for 8 KB, dwarfing the
    math it feeds).

Toolchain workarounds (this container's walrus build):
  - EVENT_SEMAPHORE_RANGE_CLEAR / DMA_QUEUE_RESET at Tile exit are
    rejected ("ISA wrong length") -> skipped (PatchedBass).
  - Sync waits on the terminal Drain are rejected ("Too many sync wait
    commands") -> moved onto EVSEM no-ops (PatchedTC).
  - Any instruction with >=2 sync waits is rejected -> waits hoisted
    onto EVSEM no-ops at BIR-JSON level (PatchedBass.to_json_bytes).
  - TensorTensorReduce opcode is unknown -> use mul + reduce instead.
"""

import json
from contextlib import ExitStack

import numpy as np

import concourse.bass as bass
import concourse.mybir as mybir
import concourse.tile as tile
from concourse.bass import SemaphoreHandle
from concourse.bass_utils import run_bass_kernel_spmd
from concourse.tile_sem_assignment import N_PROCS
from concourse.vector_clock import ScopedClock, VectorClock

SEQ = 32768
HID = 1024
NCORES = 8
SHARD = SEQ // NCORES  # 4096
P = 128  # SBUF partitions
TCOLS = SHARD // P  # 32 energy columns per core; s_local = p*TCOLS + t
TPD = 4  # seq-columns per DMA: tile = [128, TPD, 1024] = 2 MB
NDMA = TCOLS // TPD
F32 = mybir.dt.float32

# test.py pokes these to get a profiled run; harness path keeps defaults.
TRACE = {"on": False}
LAST_RESULTS = {}

MAX_WAITS_PER_INST = 1  # this walrus rejects >=2 sync waits on an instruction
WAITS_PER_EVSEM = 2


def _hoist_excess_waits(bir: dict) -> dict:
    """Move sync waits of any instruction carrying more than
    MAX_WAITS_PER_INST onto EVSEM no-ops inserted right before it on the
    same engine queue (in-order execution preserves semantics)."""
    for func in bir.get("functions", []):
        for block in func.get("blocks", []):
            new_insts = []
            for inst in block.get("instructions", []):
                si = inst.get("sync_info") or {}
                waits = si.get("on_wait") or []
                if (
                    len(waits) > MAX_WAITS_PER_INST
                    and inst.get("opcode") != "EventSemaphore"
                ):
                    for k in range(0, len(waits), WAITS_PER_EVSEM):
                        chunk = waits[k : k + WAITS_PER_EVSEM]
                        nop = {
                            "engine": inst["engine"],
                            "ins": [],
                            "outs": [],
                            "name": f"{inst['name']}-hoist{k}",
                            "opcode": "EventSemaphore",
                            "sync_info": {
                                "on_update": [
                                    {
                                        "ant_name": chunk[0]["ant_name"],
                                        "id": chunk[0]["id"],
                                        "sync_type": "semaphore",
                                        "update_mode": "sem-add-imm",
                                        "update_value": 0,
                                    }
                                ],
                                "on_wait": chunk,
                            },
                        }
                        if "debug" in inst:
                            nop["debug"] = inst["debug"]
                        new_insts.append(nop)
                    si["on_wait"] = []
                new_insts.append(inst)
            block["instructions"] = new_insts
    return bir


class PatchedBass(bass.Bass):
    """See module docstring: skips the unsupported end-of-kernel semaphore
    RANGE_CLEAR/DMA_RESET instructions and hoists excess sync waits at
    serialization time."""

    def clear_and_free_semaphores(self, sems):
        if not sems:
            return
        sem_nums = [s.num if isinstance(s, SemaphoreHandle) else s for s in sems]
        self._state.prepend_free_semaphores(sem_nums)
        for poison_set in self._tile_sem_poison_stack:
            poison_set.update(sem_nums)

    def to_json_bytes(self):
        raw = super().to_json_bytes()
        bir = json.loads(raw)
        bir = _hoist_excess_waits(bir)
        return json.dumps(bir).encode()


class PatchedTC(tile.TileContext):
    """Move the terminal waits off the Drain (rejected by this walrus) onto
    chunked EVSEM no-ops on the sync queue; in-order execution then fences
    the wait-free Drain behind them."""

    def _drain_and_barrier(self, tick_clock, wait_clock):
        nc = self.nc
        gc = tick_clock.global_clock
        sems = list(self.sems.allocated().values())
        if sems:
            dummy = sems[0]
            procs = [p for p in range(N_PROCS) if gc[p] > 0]
            for i in range(0, len(procs), WAITS_PER_EVSEM):
                chunk = procs[i : i + WAITS_PER_EVSEM]
                part = VectorClock(
                    [gc[p] if p in chunk else 0 for p in range(N_PROCS)]
                )
                nop = nc.sync.sem_inc(dummy, 0)
                wait_clock.add_sem_waits(nop.ins, ScopedClock({None: part}))
        nc.sync.drain()
        popped = nc._tile_sem_poison_stack.pop()
        assert popped is self._sem_poison
        nc.clear_and_free_semaphores(list(self.sems.allocated().values()))


def _build_nc() -> bass.Bass:
    nc = PatchedBass(
        trn_type="TRN2",
        target_bir_lowering=False,
        debug=False,
        num_devices=NCORES,
    )
    enc = nc.dram_tensor("enc", [SHARD, HID], F32, kind="ExternalInput")
    vin = nc.dram_tensor("vin", [P, HID], F32, kind="ExternalInput")
    # single fused output: per partition 32 exp values + (m, s) stats
    out_all = nc.dram_tensor("out_all", [P * (TCOLS + 2)], F32, kind="ExternalOutput")

    # s_local = p*TCOLS + t  ->  view enc as [p, t, h]
    enc_v = enc.ap().rearrange("(p t) h -> p t h", t=TCOLS)
    out_v = out_all.ap().rearrange("(p f) -> p f", f=TCOLS + 2)

    H2 = HID // 2
    NSPLIT = 4  # leading columns computed in halves so DVE starts sooner

    with PatchedTC(nc) as tc, ExitStack() as ctx:
        # single-column 512 KB loads on the in-order HWDGE FIFO; 24 resident
        # slots (recycling only gates loads >24 columns ahead of compute).
        loads = ctx.enter_context(tc.tile_pool(name="loads", bufs=24))
        scratch = ctx.enter_context(tc.tile_pool(name="scratch", bufs=10))
        dead = ctx.enter_context(tc.tile_pool(name="dead", bufs=6))
        singles = ctx.enter_context(tc.tile_pool(name="singles", bufs=1))

        # fused work tile: energies, exp values and stats side by side
        work = singles.tile([P, TCOLS + 2], F32)
        e_sbuf = work[:, 0:TCOLS]

        v_rep = singles.tile([P, HID], F32)

        def reduce_col(col, prod, e_col):
            if col in (3, 7, 11):
                nc.vector.reduce_sum(e_col, prod, axis=mybir.AxisListType.X)
            else:
                sink = dead.tile([P, HID], F32)
                nc.scalar.activation(
                    out=sink,
                    in_=prod,
                    func=mybir.ActivationFunctionType.Copy,
                    accum_out=e_col,
                )

        def do_col(col, col_ap):
            prod = scratch.tile([P, HID], F32)
            nc.vector.tensor_mul(prod, col_ap, v_rep)
            reduce_col(col, prod, e_sbuf[:, col : col + 1])

        # ---- ramp: v and the first NSPLIT columns stream in halves ----
        nc.sync.dma_start(out=v_rep[:, 0:H2], in_=vin.ap()[:, 0:H2])
        half_tiles = []
        for cc in range(NSPLIT):
            ct = loads.tile([P, 1, HID], F32, tag="cols")
            half_tiles.append(ct)
            nc.sync.dma_start(out=ct[:, :, 0:H2], in_=enc_v[:, cc : cc + 1, 0:H2])
        nc.sync.dma_start(out=v_rep[:, H2:HID], in_=vin.ap()[:, H2:HID])
        for cc in range(NSPLIT):
            nc.sync.dma_start(
                out=half_tiles[cc][:, :, H2:HID], in_=enc_v[:, cc : cc + 1, H2:HID]
            )
        for cc in range(NSPLIT):
            pa = scratch.tile([P, H2], F32, tag="half")
            nc.vector.tensor_mul(pa, half_tiles[cc][:, 0, 0:H2], v_rep[:, 0:H2])
            ea = singles.tile([P, 1], F32, tag=f"ea{cc}")
            nc.vector.reduce_sum(ea, pa, axis=mybir.AxisListType.X)
            pb = scratch.tile([P, H2], F32, tag="half")
            nc.vector.tensor_mul(pb, half_tiles[cc][:, 0, H2:HID], v_rep[:, H2:HID])
            eb = dead.tile([P, H2], F32, tag="ebsink")
            nc.scalar.activation(
                out=eb,
                in_=pb,
                func=mybir.ActivationFunctionType.Copy,
                accum_out=work[:, TCOLS : TCOLS + 1],  # borrow as temp
            )
            nc.vector.tensor_add(
                e_sbuf[:, cc : cc + 1], ea, work[:, TCOLS : TCOLS + 1]
            )

        # ---- steady state: full single-column loads ----
        for col in range(NSPLIT, TCOLS):
            col_tile = loads.tile([P, 1, HID], F32, tag="cols")
            nc.sync.dma_start(out=col_tile, in_=enc_v[:, col : col + 1, :])
            do_col(col, col_tile[:, 0, :])

        # ---- per-partition softmax stats + exp ----
        m_col = work[:, TCOLS : TCOLS + 1]
        s_col = work[:, TCOLS + 1 : TCOLS + 2]
        neg_m = singles.tile([P, 1], F32)
        exp_pp = singles.tile([P, TCOLS], F32)
        nc.vector.reduce_max(m_col, e_sbuf, axis=mybir.AxisListType.X)
        nc.scalar.mul(neg_m, m_col, -1.0)
        nc.scalar.activation(
            out=exp_pp,
            in_=e_sbuf,
            func=mybir.ActivationFunctionType.Exp,
            bias=neg_m,
            scale=1.0,
            accum_out=s_col,
        )
        nc.vector.tensor_copy(work[:, 0:TCOLS], exp_pp)
        nc.sync.dma_start(out=out_v, in_=work)

    return nc


_NC_CACHE = {}


def _get_nc() -> bass.Bass:
    if "nc" not in _NC_CACHE:
        _NC_CACHE["nc"] = _build_nc()
    return _NC_CACHE["nc"]


def kernel(hidden, encoder_outputs, W, b) -> np.ndarray:
    hidden = np.asarray(hidden, dtype=np.float32)
    encoder_outputs = np.ascontiguousarray(
        np.asarray(encoder_outputs, dtype=np.float32)
    )
    W = np.asarray(W, dtype=np.float32)

    # v = W.T @ h in f64 (tiny); b@h cancels in the softmax.
    h = hidden.reshape(-1).astype(np.float64)
    v = (W.astype(np.float64).T @ h).astype(np.float32)
    v_rep_host = np.ascontiguousarray(np.broadcast_to(v, (P, HID)))

    in_maps = [
        {
            "enc": np.ascontiguousarray(
                encoder_outputs[c * SHARD : (c + 1) * SHARD]
            ),
            "vin": v_rep_host,
        }
        for c in range(NCORES)
    ]

    nc = _get_nc()
    res = run_bass_kernel_spmd(
        nc,
        in_maps,
        core_ids=list(range(NCORES)),
        trace=TRACE["on"],
    )
    LAST_RESULTS["res"] = res

    # ---- unshard + global softmax combine (tiny: 2*1024 stats floats) ----
    allout = np.stack(
        [res.results[c]["out_all"].reshape(P, TCOLS + 2) for c in range(NCORES)]
    )  # [C, P, 34]
    exp_pp = allout[:, :, :TCOLS]  # s_global = c*SHARD + p*TCOLS + t
    m = allout[:, :, TCOLS].astype(np.float64)  # [C, P]
    s = allout[:, :, TCOLS + 1].astype(np.float64)
    gmax = m.max()
    gsum = float((s * np.exp(m - gmax)).sum())
    w = (np.exp(m - gmax) / gsum)[:, :, None]  # [C, P, 1]
    attn = (exp_pp.astype(np.float64) * w).astype(np.float32)
    return attn.reshape(1, 1, SEQ)
